# revision 6
# baseline (speedup 1.0000x reference)
"""ExpanderConv2d as a Bass/Tile kernel for Trainium2, data-parallel over batch
across 8 NeuronCores.

Reference op: y = conv2d(x, weight * mask), N=32, C=256->256, 56x56, k=3,
stride 1, pad 1.

v2: 1D Winograd F(4,3) along W.  Each output row-quad of 4 horizontal outputs
costs 6 multiplies instead of 12, so the PE streams 2/3 the columns of the
direct method's 9-tap formulation... concretely: per core the direct kernel
streams 451k columns (188us at 1 col/cycle); this kernel streams 226k
(94us), with the Winograd input transform on DVE (+ACT phase split) and the
output transform on DVE+GpSimd, all hidden under the matmul stream.

Structure per (icc, img): x [128,56,56] fp16 -> phase-split (ACT) into 4
column-phase planes [58 rows, 15 blocks] -> V[m] (DVE, 14 fused ops) for the
6 Winograd components -> GEMMs M[m] = sum_{ky,icc} W'[m,ky,icc]^T V[m]
(contraction 128ic x 3ky x 2icc into PSUM fp32, moving dim = 28 rows x 14
tiles = 392) -> ACT evicts M to SBUF fp16 -> output transform y = At M
(DVE contiguous intermediates, GpSimd strided final writes) -> fp16 y DMA.

Sharding: batch 32 -> 4 images per core; the transformed masked weight
(2.4 MB fp16, 72 [128x128] tiles) is replicated to every core.
Accuracy: 1D F(4,3) in fp16 measured 1.55e-3 scale-relative error vs the
fp32 reference (tolerance 2e-2).
"""

import numpy as np

N_CORES = 8
IMG_PER_CORE = 4
C = 256
H = 56
TX = 14          # winograd tiles per row (4 outputs each)
PHB = 15         # phase-plane blocks (E0/E1 need 15, E2/E3 use 14)
VR = 58          # V rows = padded rows
NW = 72          # weight tiles: occ(2) x m(6) x ky(3) x icc(2)


def _split_waits(nc, max_waits=1):
    """walrus in this container rejects instructions carrying more than one
    semaphore wait ("Too many sync wait commands").  Hoist the extra waits onto
    injected single-wait NoOps on the same engine just before the instruction —
    sem waits block the engine, so a chain of single waits is equivalent."""
    import concourse.mybir as mybir

    for f in nc.m.functions:
        for blk in f.blocks:
            out = []
            changed = False
            for inst in blk.instructions:
                si = inst.sync_info
                if si and si.on_wait and len(si.on_wait) > max_waits:
                    waits = list(si.on_wait)
                    extra, keep = waits[:-max_waits], waits[-max_waits:]
                    for j, w in enumerate(extra):
                        out.append(
                            mybir.InstNoOp(
                                name=f"{inst.name}-w{j}",
                                engine=inst.engine,
                                ins=[],
                                outs=[],
                                sync_info=mybir.SyncInfo(on_wait=[w], on_update=[]),
                                bass_nofuse=True,
                            )
                        )
                    si.on_wait = keep
                    changed = True
                out.append(inst)
            if changed:
                blk.instructions = out


def _build_nc():
    import concourse.bass as bass
    import concourse.mybir as mybir
    from concourse.tile import TileContext

    f32 = mybir.dt.float32
    f16 = mybir.dt.float16
    ADD = mybir.AluOpType.add
    SUB = mybir.AluOpType.subtract
    MUL = mybir.AluOpType.mult

    nc = bass.Bass("TRN2", target_bir_lowering=False, debug=False)
    x_d = nc.dram_tensor("x", [IMG_PER_CORE, C, H, H], f16, kind="ExternalInput").ap()
    w_d = nc.dram_tensor("w", [128, NW * 128], f16, kind="ExternalInput").ap()
    y_d = nc.dram_tensor("y", [IMG_PER_CORE, C, H, H], f16, kind="ExternalOutput").ap()

    with TileContext(nc) as tc:
        with (
            tc.tile_pool(name="wpool", bufs=1) as wp,
            tc.tile_pool(name="xpool", bufs=1) as xp,
            tc.tile_pool(name="psum", bufs=8, space="PSUM") as pp,
            tc.tile_pool(name="msb", bufs=1) as mp,
            tc.tile_pool(name="ypool", bufs=1) as yp,
            tc.tile_pool(name="scp", bufs=1) as scp,
        ):
            w_sb = wp.tile([128, NW * 128], f16, name="w_sb", tag="w_sb")
            # First GEMM group needs tiles 0..5 ((occ0, m0) x ky x icc);
            # stage those first so matmuls can start early.
            wq = 0
            for wn in (6, 30, 36):
                nc.scalar.dma_start(
                    out=w_sb[:, wq * 128 : (wq + wn) * 128],
                    in_=w_d[:, wq * 128 : (wq + wn) * 128],
                )
                wq += wn

            # Warm the PE clock gate (HAM) with throwaway matmuls on scratch
            # data while the first input/weight DMAs are in flight.
            warm = wp.tile([128, 392], f16, name="warm", tag="warm")
            nc.vector.memset(warm[:], 0.0)
            N_WARM = 10
            for i in range(N_WARM):
                warm_ps = pp.tile([128, 28, TX], f32, name="mt", tag="mt")
                nc.tensor.matmul(
                    warm_ps[:],
                    warm[:, :128],
                    warm[:, :392],
                    start=True,
                    stop=True,
                )

            # Per (icc, ping/pong): raw input, phase planes, V components.
            xrs = [
                [xp.tile([128, H, H], f16, name=f"xr{i}{b}", tag=f"xr{i}{b}") for b in range(2)]
                for i in range(2)
            ]
            phs = [
                [xp.tile([128, 4, VR, PHB], f16, name=f"ph{i}{b}", tag=f"ph{i}{b}") for b in range(2)]
                for i in range(2)
            ]
            vts = [
                [xp.tile([128, 6, VR, TX], f16, name=f"vt{i}{b}", tag=f"vt{i}{b}") for b in range(2)]
                for i in range(2)
            ]
            # Zero the padding cells of the phase planes once; DMA+split only
            # ever write interior rows/blocks, so they stay zero.
            for i in range(2):
                for b in range(2):
                    ph = phs[i][b]
                    nc.gpsimd.memset(ph[:, :, 0, :], 0.0)        # top pad row
                    nc.gpsimd.memset(ph[:, :, VR - 1, :], 0.0)   # bottom pad row
                    nc.gpsimd.memset(ph[:, 0, :, 0], 0.0)        # left pad col (E0 blk 0)
                    nc.gpsimd.memset(ph[:, 1, :, PHB - 1], 0.0)  # right pad col (E1 blk 14)

            sc = scp.tile([128, 8, 30, TX], f16, name="sc", tag="sc")
            msbs = [mp.tile([128, 6, H, TX], f16, name=f"m{b}", tag=f"m{b}") for b in range(2)]
            sc2s = [scp.tile([128, 10, 28, TX], f16, name=f"s2{b}", tag=f"s2{b}") for b in range(2)]
            ys = [yp.tile([128, H, H], f16, name=f"y{b}", tag=f"y{b}") for b in range(2)]

            # Row halves: half A covers V rows 0..29 (x rows 0..28), half B
            # V rows 30..57 (x rows 29..55).
            HALves = [(0, 30, 0, 29), (30, VR, 29, H)]

            def emit_input_stage(img):
                pg = img % 2
                for icc in range(2):
                    xr = xrs[icc][pg]
                    for (_, _, xa, xb) in HALves:
                        nc.sync.dma_start(
                            out=xr[:, xa:xb, :],
                            in_=x_d[img, icc * 128 : (icc + 1) * 128, xa:xb, :],
                        )
                for icc in range(2):
                    xr, ph = xrs[icc][pg], phs[icc][pg]
                    for (va, vb, xa, xb) in HALves:
                        # phase p holds padded col c = 4*blk + p; data col w = c-1.
                        r0 = va + 1 if va == 0 else va
                        r1 = vb if vb != VR else VR - 1
                        nc.scalar.copy(out=ph[:, 0, r0:r1, 1:PHB], in_=xr[:, xa:xb, 3:H:4])
                        nc.scalar.copy(out=ph[:, 1, r0:r1, 0:14], in_=xr[:, xa:xb, 0:H:4])
                        nc.scalar.copy(out=ph[:, 2, r0:r1, 0:14], in_=xr[:, xa:xb, 1:H:4])
                        nc.scalar.copy(out=ph[:, 3, r0:r1, 0:14], in_=xr[:, xa:xb, 2:H:4])
                for icc in range(2):
                    ph, vt = phs[icc][pg], vts[icc][pg]
                    for (va, vb, _, _) in HALves:
                        n = vb - va
                        q0 = ph[:, 0, va:vb, 0:TX]
                        q1 = ph[:, 1, va:vb, 0:TX]
                        q2 = ph[:, 2, va:vb, 0:TX]
                        q3 = ph[:, 3, va:vb, 0:TX]
                        q4 = ph[:, 0, va:vb, 1:PHB]
                        q5 = ph[:, 1, va:vb, 1:PHB]
                        t0 = sc[:, 0, :n, :]
                        A = sc[:, 1, :n, :]
                        B = sc[:, 2, :n, :]
                        Cc = sc[:, 3, :n, :]
                        D = sc[:, 4, :n, :]
                        E = sc[:, 5, :n, :]
                        F = sc[:, 6, :n, :]
                        t2 = sc[:, 7, :n, :]
                        v = lambda m: vt[:, m, va:vb, :]
                        nc.vector.scalar_tensor_tensor(t0, q0, 4.0, q4, MUL, ADD)
                        nc.vector.scalar_tensor_tensor(v(0), q2, -5.0, t0, MUL, ADD)
                        nc.vector.tensor_tensor(A, q1, q2, ADD)
                        nc.vector.tensor_tensor(B, q3, q4, ADD)
                        nc.vector.scalar_tensor_tensor(v(1), A, -4.0, B, MUL, ADD)
                        nc.vector.tensor_tensor(Cc, q1, q2, SUB)
                        nc.vector.tensor_tensor(D, q3, q4, SUB)
                        nc.vector.scalar_tensor_tensor(v(2), Cc, 4.0, D, MUL, SUB)
                        nc.vector.tensor_tensor(E, q1, q3, SUB)
                        nc.vector.tensor_tensor(F, q4, q2, SUB)
                        nc.vector.scalar_tensor_tensor(v(3), E, -2.0, F, MUL, ADD)
                        nc.vector.scalar_tensor_tensor(v(4), E, 2.0, F, MUL, ADD)
                        nc.vector.scalar_tensor_tensor(t2, q1, 4.0, q5, MUL, ADD)
                        nc.vector.scalar_tensor_tensor(v(5), q3, -5.0, t2, MUL, ADD)

            def emit_compute_stage(img):
                pg = img % 2
                for occ in range(2):
                    msb = msbs[(img * 2 + occ) % 2]
                    yt = ys[(img * 2 + occ) % 2]
                    for chunk in range(2):
                        c0 = chunk * 28
                        for m in range(6):
                            mt = pp.tile([128, 28, TX], f32, name="mt", tag="mt")
                            t = 0
                            for ky in range(3):
                                for icc in range(2):
                                    widx = ((occ * 6 + m) * 3 + ky) * 2 + icc
                                    nc.tensor.matmul(
                                        mt[:],
                                        w_sb[:, widx * 128 : (widx + 1) * 128],
                                        vts[icc][pg][:, m, c0 + ky : c0 + ky + 28, :],
                                        start=(t == 0),
                                        stop=(t == 5),
                                    )
                                    t += 1
                            ev = (mt, msb[:, m, c0 : c0 + 28, :])
                            # First chunk of the first occ: evict immediately
                            # (frees PSUM banks for the rolling allocation);
                            # later ones also go right away.
                            nc.scalar.copy(out=ev[1], in_=ev[0][:])
                        # output transform for this (occ, chunk)
                        s2 = sc2s[(occ * 2 + chunk) % 2]
                        ms = lambda m: msb[:, m, c0 : c0 + 28, :]
                        I_ = s2[:, 0, :, :]
                        J_ = s2[:, 1, :, :]
                        IJ = s2[:, 2, :, :]
                        G2 = s2[:, 3, :, :]
                        H2 = s2[:, 4, :, :]
                        y3t = s2[:, 5, :, :]
                        yv = [s2[:, 6 + v, :, :] for v in range(4)]
                        nc.vector.tensor_tensor(I_, ms(1), ms(2), ADD)
                        nc.vector.tensor_tensor(J_, ms(3), ms(4), ADD)
                        nc.vector.tensor_tensor(IJ, I_, J_, ADD)
                        nc.vector.tensor_tensor(G2, ms(1), ms(2), SUB)
                        nc.vector.tensor_tensor(H2, ms(3), ms(4), SUB)
                        nc.vector.scalar_tensor_tensor(y3t, H2, 8.0, G2, MUL, ADD)
                        nc.vector.tensor_tensor(yv[0], IJ, ms(0), ADD)
                        nc.vector.scalar_tensor_tensor(yv[1], H2, 2.0, G2, MUL, ADD)
                        nc.vector.scalar_tensor_tensor(yv[2], J_, 4.0, I_, MUL, ADD)
                        nc.vector.tensor_tensor(yv[3], y3t, ms(5), ADD)
                        for v in range(4):
                            nc.gpsimd.tensor_copy(
                                out=yt[:, c0 : c0 + 28, v:H:4], in_=yv[v]
                            )
                        nc.sync.dma_start(
                            out=y_d[img, occ * 128 : (occ + 1) * 128, c0 : c0 + 28, :],
                            in_=yt[:, c0 : c0 + 28, :],
                        )

            # Software pipeline: input stage for img runs one iteration ahead
            # of its compute stage, so DVE/ACT work hides under the PE stream.
            emit_input_stage(0)
            for img in range(1, IMG_PER_CORE):
                emit_input_stage(img)
                emit_compute_stage(img - 1)
            emit_compute_stage(IMG_PER_CORE - 1)

    _split_waits(nc)
    return nc


def _prep_weight(weight: np.ndarray, mask: np.ndarray) -> np.ndarray:
    """[OC, IC, K, K] masked weight -> Winograd-transformed lhsT tiles
    [128ic, (occ,m,ky,icc)*128oc]."""
    G = np.array(
        [
            [1 / 4, 0, 0],
            [-1 / 6, -1 / 6, -1 / 6],
            [-1 / 6, 1 / 6, -1 / 6],
            [1 / 24, 1 / 12, 1 / 6],
            [1 / 24, -1 / 12, 1 / 6],
            [0, 0, 1],
        ],
        np.float32,
    )
    wm = (weight * mask).astype(np.float32)                  # [oc, ic, ky, kx]
    wp = np.einsum("mx,oikx->moik", G, wm)                   # [m, oc, ic, ky]
    t = wp.reshape(6, 2, 128, 2, 128, 3)                     # [m, occ, oc, icc, ic, ky]
    t = t.transpose(4, 1, 0, 5, 3, 2)                        # [ic, occ, m, ky, icc, oc]
    return np.ascontiguousarray(t.reshape(128, NW * 128).astype(np.float16))


def kernel(x: np.ndarray, weight: np.ndarray, mask: np.ndarray) -> np.ndarray:
    from concourse.bass_utils import run_bass_kernel_spmd

    x = np.asarray(x, dtype=np.float32)
    x16 = np.ascontiguousarray(x.astype(np.float16))
    w_host = _prep_weight(np.asarray(weight), np.asarray(mask))

    nc = _build_nc()
    in_maps = [
        {
            "x": np.ascontiguousarray(x16[c * IMG_PER_CORE : (c + 1) * IMG_PER_CORE]),
            "w": w_host,
        }
        for c in range(N_CORES)
    ]
    res = run_bass_kernel_spmd(nc, in_maps, core_ids=list(range(N_CORES)))
    out = np.empty_like(x)
    for c in range(N_CORES):
        out[c * IMG_PER_CORE : (c + 1) * IMG_PER_CORE] = res.results[c]["y"].astype(
            np.float32
        )
    return out


# revision 7
# speedup vs baseline: 1.1982x; 1.1982x over previous
"""ExpanderConv2d as a Bass/Tile kernel for Trainium2, data-parallel over batch
across 8 NeuronCores.

Reference op: y = conv2d(x, weight * mask), N=32, C=256->256, 56x56, k=3,
stride 1, pad 1.

v3: 1D Winograd F(4,3) along W.  Each quad of 4 horizontal outputs costs 6
multiplies instead of 12, so the PE streams 2/3 the columns of the direct
9-tap formulation: 226k columns/core (~94us at 1 col/cycle) vs 451k (~188us).

Pipeline per (icc, img): x [128,56,56] fp16 -> ACT phase-split into 4
column-phase planes [58 rows, 15 blocks] -> V[m] (DVE/GpSimd, 15 fused ops)
for the 6 Winograd components -> GEMMs M[m] = sum_{ky,icc} W'[m,ky,icc]^T
V[m] (PSUM fp32, moving dim = 28 rows x 14 tiles = 392) -> ACT evicts M to
SBUF fp16 -> output transform y[v] = At M (DVE + GpSimd) into per-phase
planes -> fp16 planar DMA out; the host interleaves the 4 phases (pure
reshape/transpose) and upcasts.

Sharding: batch 32 -> 4 images per core; the transformed masked weight
(2.4 MB fp16, 72 [128x128] tiles) is replicated to every core.
Accuracy: measured 4.1e-3 scale-relative error vs fp32 reference (tol 2e-2).
"""

import numpy as np

N_CORES = 8
IMG_PER_CORE = 4
C = 256
H = 56
TX = 14          # winograd tiles per row (4 outputs each)
PHB = 15         # phase-plane blocks (E0/E1 need 15, E2/E3 use 14)
VR = 58          # V rows = padded rows
NW = 72          # weight tiles: occ(2) x m(6) x ky(3) x icc(2)


def _split_waits(nc, max_waits=1):
    """walrus in this container rejects instructions carrying more than one
    semaphore wait ("Too many sync wait commands").  Hoist the extra waits onto
    injected single-wait NoOps on the same engine just before the instruction —
    sem waits block the engine, so a chain of single waits is equivalent."""
    import concourse.mybir as mybir

    for f in nc.m.functions:
        for blk in f.blocks:
            out = []
            changed = False
            for inst in blk.instructions:
                si = inst.sync_info
                if si and si.on_wait and len(si.on_wait) > max_waits:
                    waits = list(si.on_wait)
                    extra, keep = waits[:-max_waits], waits[-max_waits:]
                    for j, w in enumerate(extra):
                        out.append(
                            mybir.InstNoOp(
                                name=f"{inst.name}-w{j}",
                                engine=inst.engine,
                                ins=[],
                                outs=[],
                                sync_info=mybir.SyncInfo(on_wait=[w], on_update=[]),
                                bass_nofuse=True,
                            )
                        )
                    si.on_wait = keep
                    changed = True
                out.append(inst)
            if changed:
                blk.instructions = out


def _build_nc():
    import concourse.bass as bass
    import concourse.mybir as mybir
    from concourse.tile import TileContext

    f32 = mybir.dt.float32
    f16 = mybir.dt.float16
    ADD = mybir.AluOpType.add
    SUB = mybir.AluOpType.subtract
    MUL = mybir.AluOpType.mult

    nc = bass.Bass("TRN2", target_bir_lowering=False, debug=False)
    x_d = nc.dram_tensor("x", [IMG_PER_CORE, C, H, H], f16, kind="ExternalInput").ap()
    w_d = nc.dram_tensor("w", [128, NW * 128], f16, kind="ExternalInput").ap()
    # y is stored phase-planar: y[img, c, v, h, tx] = out[img, c, h, 4*tx+v]
    y_d = nc.dram_tensor("y", [IMG_PER_CORE, C, 4, H, TX], f16, kind="ExternalOutput").ap()

    with TileContext(nc) as tc:
        with (
            tc.tile_pool(name="wpool", bufs=1) as wp,
            tc.tile_pool(name="xpool", bufs=1) as xp,
            tc.tile_pool(name="psum", bufs=8, space="PSUM") as pp,
            tc.tile_pool(name="msb", bufs=1) as mp,
            tc.tile_pool(name="scp", bufs=1) as scp,
        ):
            w_sb = wp.tile([128, NW * 128], f16, name="w_sb", tag="w_sb")
            # First GEMM group needs tiles 0..5 ((occ0, m0) x ky x icc);
            # stage those first so matmuls can start early.
            wq = 0
            for wn in (6, 30, 36):
                nc.scalar.dma_start(
                    out=w_sb[:, wq * 128 : (wq + wn) * 128],
                    in_=w_d[:, wq * 128 : (wq + wn) * 128],
                )
                wq += wn

            # Warm the PE clock gate (HAM) with throwaway matmuls on scratch
            # data while the first input/weight DMAs are in flight.
            warm = wp.tile([128, 392], f16, name="warm", tag="warm")
            nc.vector.memset(warm[:], 0.0)
            N_WARM = 10
            for i in range(N_WARM):
                warm_ps = pp.tile([128, 28, TX], f32, name="mt", tag="mt")
                nc.tensor.matmul(
                    warm_ps[:], warm[:, :128], warm[:, :392], start=True, stop=True
                )

            # Per (icc, ping/pong): raw input, phase planes, V components.
            xrs = [
                [xp.tile([128, H, H], f16, name=f"xr{i}{b}", tag=f"xr{i}{b}") for b in range(2)]
                for i in range(2)
            ]
            phs = [
                [xp.tile([128, 4, VR, PHB], f16, name=f"ph{i}{b}", tag=f"ph{i}{b}") for b in range(2)]
                for i in range(2)
            ]
            vts = [
                [xp.tile([128, 6, VR, TX], f16, name=f"vt{i}{b}", tag=f"vt{i}{b}") for b in range(2)]
                for i in range(2)
            ]
            # Zero the padding cells of the phase planes once; DMA+split only
            # ever write interior rows/blocks, so they stay zero.
            for i in range(2):
                for b in range(2):
                    ph = phs[i][b]
                    nc.gpsimd.memset(ph[:, :, 0, :], 0.0)        # top pad row
                    nc.gpsimd.memset(ph[:, :, VR - 1, :], 0.0)   # bottom pad row
                    nc.gpsimd.memset(ph[:, 0, :, 0], 0.0)        # left pad col (E0 blk 0)
                    nc.gpsimd.memset(ph[:, 1, :, PHB - 1], 0.0)  # right pad col (E1 blk 14)

            # V scratch: separate tiles for the DVE- and GpSimd-computed
            # subexpressions so the engines never falsely serialize.
            # slabs: 0:A 1:B 2:C 3:D 4:Es 5:F 6:E2s 7:u 8:w5
            sc_d = scp.tile([128, 9, VR, TX], f16, name="sc_d", tag="sc_d")
            sc_g = scp.tile([128, 9, VR, TX], f16, name="sc_g", tag="sc_g")
            msbs = [mp.tile([128, 6, H, TX], f16, name=f"m{b}", tag=f"m{b}") for b in range(2)]
            sc2s = [scp.tile([128, 10, 28, TX], f16, name=f"s2{b}", tag=f"s2{b}") for b in range(2)]

            def emit_v_ops(icc, img, va, vb):
                """V-transform rows va..vb for one (icc, img).  The 11 pure
                tensor_tensor ops can run on either engine; for icc1 the six
                subexpression TTs go to GpSimd to offload DVE."""
                pg = img % 2
                ph, vt, sc = phs[icc][pg], vts[icc][pg], (sc_g if icc else sc_d)
                q0 = ph[:, 0, va:vb, 0:TX]
                q1 = ph[:, 1, va:vb, 0:TX]
                q2 = ph[:, 2, va:vb, 0:TX]
                q3 = ph[:, 3, va:vb, 0:TX]
                q4 = ph[:, 0, va:vb, 1:PHB]
                q5 = ph[:, 1, va:vb, 1:PHB]
                A = sc[:, 0, va:vb, :]
                B = sc[:, 1, va:vb, :]
                Cc = sc[:, 2, va:vb, :]
                D = sc[:, 3, va:vb, :]
                Es = sc[:, 4, va:vb, :]
                F = sc[:, 5, va:vb, :]
                E2s = sc[:, 6, va:vb, :]
                u = sc[:, 7, va:vb, :]
                w5 = sc[:, 8, va:vb, :]
                v = lambda m: vt[:, m, va:vb, :]
                eng = nc.gpsimd if icc == 1 else nc.vector
                eng.tensor_tensor(A, q1, q2, ADD)
                eng.tensor_tensor(B, q3, q4, ADD)
                eng.tensor_tensor(Cc, q1, q2, SUB)
                eng.tensor_tensor(D, q3, q4, SUB)
                eng.tensor_tensor(Es, q1, q3, SUB)
                eng.tensor_tensor(F, q4, q2, SUB)
                nc.vector.tensor_tensor(E2s, Es, Es, ADD)
                nc.vector.tensor_tensor(u, q0, q2, SUB)
                nc.vector.tensor_tensor(w5, q5, q3, SUB)
                nc.vector.scalar_tensor_tensor(v(0), u, 4.0, F, MUL, ADD)
                nc.vector.scalar_tensor_tensor(v(1), A, -4.0, B, MUL, ADD)
                nc.vector.scalar_tensor_tensor(v(2), Cc, 4.0, D, MUL, SUB)
                nc.vector.tensor_tensor(v(3), F, E2s, SUB)
                nc.vector.tensor_tensor(v(4), F, E2s, ADD)
                nc.vector.scalar_tensor_tensor(v(5), Es, 4.0, w5, MUL, ADD)

            def emit_input_stage(img, halved=False):
                pg = img % 2
                # Row halves: half A covers V rows 0..29 (x rows 0..28),
                # half B V rows 30..57 (x rows 29..55).
                halves = [(0, 30, 0, 29), (30, VR, 29, H)]
                for icc in range(2):
                    xr = xrs[icc][pg]
                    for (_, _, xa, xb) in halves:
                        nc.sync.dma_start(
                            out=xr[:, xa:xb, :],
                            in_=x_d[img, icc * 128 : (icc + 1) * 128, xa:xb, :],
                        )
                for icc in range(2):
                    xr, ph = xrs[icc][pg], phs[icc][pg]
                    for (va, vb, xa, xb) in halves:
                        # phase p holds padded col c = 4*blk + p; data col w = c-1.
                        r0 = va + 1 if va == 0 else va
                        r1 = vb if vb != VR else VR - 1
                        nc.scalar.copy(out=ph[:, 0, r0:r1, 1:PHB], in_=xr[:, xa:xb, 3:H:4])
                        nc.scalar.copy(out=ph[:, 1, r0:r1, 0:14], in_=xr[:, xa:xb, 0:H:4])
                        nc.scalar.copy(out=ph[:, 2, r0:r1, 0:14], in_=xr[:, xa:xb, 1:H:4])
                        nc.scalar.copy(out=ph[:, 3, r0:r1, 0:14], in_=xr[:, xa:xb, 2:H:4])
                for icc in range(2):
                    if halved:
                        for (va, vb, _, _) in halves:
                            emit_v_ops(icc, img, va, vb)
                    else:
                        emit_v_ops(icc, img, 0, VR)

            def emit_compute_stage(img):
                pg = img % 2
                for occ in range(2):
                    msb = msbs[(img * 2 + occ) % 2]
                    for chunk in range(2):
                        c0 = chunk * 28
                        for m in range(6):
                            mt = pp.tile([128, 28, TX], f32, name="mt", tag="mt")
                            t = 0
                            for ky in range(3):
                                for icc in range(2):
                                    widx = ((occ * 6 + m) * 3 + ky) * 2 + icc
                                    nc.tensor.matmul(
                                        mt[:],
                                        w_sb[:, widx * 128 : (widx + 1) * 128],
                                        vts[icc][pg][:, m, c0 + ky : c0 + ky + 28, :],
                                        start=(t == 0),
                                        stop=(t == 5),
                                    )
                                    t += 1
                            nc.scalar.copy(out=msb[:, m, c0 : c0 + 28, :], in_=mt[:])
                        # output transform for this (occ, chunk): into
                        # per-phase planes, DMA'd planar (host interleaves).
                        s2 = sc2s[(occ * 2 + chunk) % 2]
                        ms = lambda m: msb[:, m, c0 : c0 + 28, :]
                        I_ = s2[:, 0, :, :]
                        J_ = s2[:, 1, :, :]
                        IJ = s2[:, 2, :, :]
                        G2 = s2[:, 3, :, :]
                        H2 = s2[:, 4, :, :]
                        y3t = s2[:, 5, :, :]
                        yv = [s2[:, 6 + v, :, :] for v in range(4)]
                        nc.vector.tensor_tensor(I_, ms(1), ms(2), ADD)
                        nc.vector.tensor_tensor(J_, ms(3), ms(4), ADD)
                        nc.gpsimd.tensor_tensor(IJ, I_, J_, ADD)
                        nc.vector.tensor_tensor(G2, ms(1), ms(2), SUB)
                        nc.vector.tensor_tensor(H2, ms(3), ms(4), SUB)
                        nc.vector.scalar_tensor_tensor(y3t, H2, 8.0, G2, MUL, ADD)
                        nc.gpsimd.tensor_tensor(yv[0], IJ, ms(0), ADD)
                        nc.vector.scalar_tensor_tensor(yv[1], H2, 2.0, G2, MUL, ADD)
                        nc.vector.scalar_tensor_tensor(yv[2], J_, 4.0, I_, MUL, ADD)
                        nc.gpsimd.tensor_tensor(yv[3], y3t, ms(5), ADD)
                        for v in range(4):
                            nc.sync.dma_start(
                                out=y_d[img, occ * 128 : (occ + 1) * 128, v, c0 : c0 + 28, :],
                                in_=yv[v],
                            )

            # Software pipeline: input stage for img runs one iteration ahead
            # of its compute stage, so DVE/ACT/GpSimd work hides under the PE
            # stream.
            emit_input_stage(0, halved=True)
            for img in range(1, IMG_PER_CORE):
                emit_input_stage(img)
                emit_compute_stage(img - 1)
            emit_compute_stage(IMG_PER_CORE - 1)

    _split_waits(nc)
    return nc


def _prep_weight(weight: np.ndarray, mask: np.ndarray) -> np.ndarray:
    """[OC, IC, K, K] masked weight -> Winograd-transformed lhsT tiles
    [128ic, (occ,m,ky,icc)*128oc]."""
    G = np.array(
        [
            [1 / 4, 0, 0],
            [-1 / 6, -1 / 6, -1 / 6],
            [-1 / 6, 1 / 6, -1 / 6],
            [1 / 24, 1 / 12, 1 / 6],
            [1 / 24, -1 / 12, 1 / 6],
            [0, 0, 1],
        ],
        np.float32,
    )
    wm = (weight * mask).astype(np.float32)                  # [oc, ic, ky, kx]
    wp = np.einsum("mx,oikx->moik", G, wm)                   # [m, oc, ic, ky]
    t = wp.reshape(6, 2, 128, 2, 128, 3)                     # [m, occ, oc, icc, ic, ky]
    t = t.transpose(4, 1, 0, 5, 3, 2)                        # [ic, occ, m, ky, icc, oc]
    return np.ascontiguousarray(t.reshape(128, NW * 128).astype(np.float16))


def kernel(x: np.ndarray, weight: np.ndarray, mask: np.ndarray) -> np.ndarray:
    from concourse.bass_utils import run_bass_kernel_spmd

    x = np.asarray(x, dtype=np.float32)
    x16 = np.ascontiguousarray(x.astype(np.float16))
    w_host = _prep_weight(np.asarray(weight), np.asarray(mask))

    nc = _build_nc()
    in_maps = [
        {
            "x": np.ascontiguousarray(x16[c * IMG_PER_CORE : (c + 1) * IMG_PER_CORE]),
            "w": w_host,
        }
        for c in range(N_CORES)
    ]
    res = run_bass_kernel_spmd(nc, in_maps, core_ids=list(range(N_CORES)))
    out = np.empty_like(x)
    for c in range(N_CORES):
        yp = res.results[c]["y"]  # [4, C, 4, 56, 14] phase-planar fp16
        yi = np.transpose(yp, (0, 1, 3, 4, 2)).reshape(IMG_PER_CORE, C, H, H)
        out[c * IMG_PER_CORE : (c + 1) * IMG_PER_CORE] = yi.astype(np.float32)
    return out
